# revision 2
# baseline (speedup 1.0000x reference)
"""Trainium2 Bass kernel for multi-head attention (B=4, S=2048, D=1024, H=16).

Sharding: 8 cores = 4-way batch x 2-way head-group (8 heads per core).
Each core computes, for its batch element b and head-group g:
  qT/kT = (W.T x.T) in transposed layout [local_hd, S] (head-pair chunks of 128
  partitions), V in normal layout [S, local_hd], scores^T = kT.T-free matmuls
  with row-packed head pairs (K=64 x2 concurrent), exp on ACT (no max
  subtraction; scores are provably O(1) for this problem), AV + denominator
  matmuls col-packed via tile_position, reciprocal_approx_fast, normalization
  fused into the PSUM->SBUF copy, then the output projection producing a
  partial [S, D] that the host sums across the 2 head-group cores + bias.
"""

import numpy as np
import ml_dtypes
from contextlib import ExitStack

BF16 = ml_dtypes.bfloat16

S = 2048          # sequence length
D = 1024          # model dim
DH = 64           # head dim
HL = 8            # local heads per core
HD = HL * DH      # 512 local output dims per core
NB = 4            # batch
SCALE = 1.0 / (DH ** 0.5)

KC = D // 128     # 8 contraction chunks for projections
MC = HD // 128    # 4 output-dim chunks (= head pairs) per core
IC = S // 512     # 4 query chunks of 512
JC = S // 128     # 16 key chunks of 128
SC = S // 128     # 16 output row chunks

_NC_CACHE = {}


def _build_nc():
    import concourse.bacc as bacc
    import concourse.tile as tile
    from concourse import mybir

    f32 = mybir.dt.float32
    bf16 = mybir.dt.bfloat16
    Exp = mybir.ActivationFunctionType.Exp

    nc = bacc.Bacc("TRN2", target_bir_lowering=False, debug=False)

    xT_d = nc.dram_tensor("xT", [D, S], bf16, kind="ExternalInput")
    wqT_d = nc.dram_tensor("wqT", [D, HD], bf16, kind="ExternalInput")
    wkT_d = nc.dram_tensor("wkT", [D, HD], bf16, kind="ExternalInput")
    wvT_d = nc.dram_tensor("wvT", [D, HD], bf16, kind="ExternalInput")
    woT_d = nc.dram_tensor("woT", [HD, D], bf16, kind="ExternalInput")
    bq_d = nc.dram_tensor("bq", [128, MC], f32, kind="ExternalInput")
    bk_d = nc.dram_tensor("bk", [128, MC], f32, kind="ExternalInput")
    bv_d = nc.dram_tensor("bv", [1, HD], f32, kind="ExternalInput")
    out_d = nc.dram_tensor("out", [S, D], f32, kind="ExternalOutput")

    with tile.TileContext(nc) as tc, ExitStack() as ctx:
        consts = ctx.enter_context(tc.tile_pool(name="consts", bufs=1))
        persist = ctx.enter_context(tc.tile_pool(name="persist", bufs=1))

        # ---- constant/weight loads ----
        xT_sb = consts.tile([128, KC, S], bf16)
        wqT_sb = consts.tile([128, KC, HD], bf16)
        wkT_sb = consts.tile([128, KC, HD], bf16)
        wvT_sb = consts.tile([128, KC, HD], bf16)
        woT_sb = consts.tile([128, MC, D], bf16)
        bq_sb = consts.tile([128, MC], f32)
        bk_sb = consts.tile([128, MC], f32)
        bvb_sb = consts.tile([128, HD], f32)   # bv broadcast across partitions
        ones_sb = consts.tile([128, 64], bf16)

        for k in range(KC):
            nc.sync.dma_start(xT_sb[:, k, :], xT_d.ap()[k * 128:(k + 1) * 128, :])
            nc.sync.dma_start(wqT_sb[:, k, :], wqT_d.ap()[k * 128:(k + 1) * 128, :])
            nc.sync.dma_start(wkT_sb[:, k, :], wkT_d.ap()[k * 128:(k + 1) * 128, :])
            nc.sync.dma_start(wvT_sb[:, k, :], wvT_d.ap()[k * 128:(k + 1) * 128, :])
        for k in range(MC):
            nc.sync.dma_start(woT_sb[:, k, :], woT_d.ap()[k * 128:(k + 1) * 128, :])
        nc.sync.dma_start(bq_sb[:], bq_d.ap())
        nc.sync.dma_start(bk_sb[:], bk_d.ap())
        # broadcast bv along partitions via 0-step AP
        import concourse.bass as bass
        bv_ap = bv_d.ap()
        bv_bcast = bass.AP(tensor=bv_ap.tensor, offset=bv_ap.offset,
                           ap=[[0, 128]] + [bv_ap.ap[-1]])
        nc.sync.dma_start(bvb_sb[:], bv_bcast)
        nc.vector.memset(ones_sb[:], 1.0)

        qT_sb = persist.tile([128, MC, S], bf16)
        kT_sb = persist.tile([128, MC, S], bf16)
        v_sb = persist.tile([128, JC, HD], bf16)
        avT_sb = persist.tile([128, MC, S], bf16)

        # ---- phase 1: projections ----
        with tc.tile_pool(name="ps1", bufs=3, space="PSUM") as ps1:
            # qT, kT: [local_hd, S] via lhsT=W^T chunks (stationary), rhs=x^T
            for m in range(MC):
                for i in range(IC):
                    isl = slice(i * 512, (i + 1) * 512)
                    msl = slice(m * 128, (m + 1) * 128)
                    psq = ps1.tile([128, 512], f32, tag="ps1")
                    for k in range(KC):
                        nc.tensor.matmul(psq[:], wqT_sb[:, k, msl],
                                         xT_sb[:, k, isl],
                                         start=(k == 0), stop=(k == KC - 1))
                    nc.vector.tensor_scalar_add(qT_sb[:, m, isl], psq[:],
                                                bq_sb[:, m:m + 1])
                    psk = ps1.tile([128, 512], f32, tag="ps1")
                    for k in range(KC):
                        nc.tensor.matmul(psk[:], wkT_sb[:, k, msl],
                                         xT_sb[:, k, isl],
                                         start=(k == 0), stop=(k == KC - 1))
                    nc.vector.tensor_scalar_add(kT_sb[:, m, isl], psk[:],
                                                bk_sb[:, m:m + 1])
            # V in normal layout [S, local_hd]: lhsT = x^T chunk, rhs = wv^T
            for t in range(JC):
                tsl = slice(t * 128, (t + 1) * 128)
                psv = ps1.tile([128, 512], f32, tag="ps1")
                for k in range(KC):
                    nc.tensor.matmul(psv[:], xT_sb[:, k, tsl], wvT_sb[:, k, :],
                                     start=(k == 0), stop=(k == KC - 1))
                nc.vector.tensor_add(v_sb[:, t, :], psv[:], bvb_sb[:])

        # ---- phase 2: attention ----
        with tc.tile_pool(name="st", bufs=2, space="PSUM") as stp, \
             tc.tile_pool(name="av", bufs=2, space="PSUM") as avp, \
             tc.tile_pool(name="dn", bufs=2, space="PSUM") as dnp, \
             tc.tile_pool(name="ep", bufs=4) as ep, \
             tc.tile_pool(name="rp", bufs=2) as rp:
            for m in range(MC):
                h0 = 2 * m
                isl_all = [slice(i * 512, (i + 1) * 512) for i in range(IC)]
                for i in range(IC):
                    isl = isl_all[i]
                    av = avp.tile([128, 512], f32, tag="av")
                    dn = dnp.tile([128, 512], f32, tag="dn")
                    for jg in range(JC // 2):
                        st_h = stp.tile([128, 2, 512], f32, tag="st")
                        st_h1 = stp.tile([128, 2, 512], f32, tag="st")
                        for jj in range(2):
                            j = jg * 2 + jj
                            jsl = slice(j * 128, (j + 1) * 128)
                            # scores^T, row-packed head pair (K=64 each)
                            nc.tensor.matmul(st_h[:, jj], kT_sb[0:64, m, jsl],
                                             qT_sb[0:64, m, isl],
                                             start=True, stop=True)
                            nc.tensor.matmul(st_h1[:, jj], kT_sb[64:128, m, jsl],
                                             qT_sb[64:128, m, isl],
                                             start=True, stop=True)
                        e_h = ep.tile([128, 2, 512], bf16, tag="e")
                        e_h1 = ep.tile([128, 2, 512], bf16, tag="e")
                        nc.scalar.activation(e_h[:], st_h[:], Exp)
                        nc.scalar.activation(e_h1[:], st_h1[:], Exp)
                        for jj in range(2):
                            j = jg * 2 + jj
                            first = (j == 0)
                            last = (j == JC - 1)
                            # AV col-packed: head h -> psum rows 0-63,
                            # head h+1 -> rows 64-127 (tile_position (0,64))
                            nc.tensor.matmul(av[0:64, :],
                                             v_sb[:, j, h0 * 64:(h0 + 1) * 64],
                                             e_h[:, jj],
                                             start=first, stop=last)
                            nc.tensor.matmul(av[64:128, :],
                                             v_sb[:, j, (h0 + 1) * 64:(h0 + 2) * 64],
                                             e_h1[:, jj],
                                             start=first, stop=last)
                            # denominators, col-packed the same way
                            nc.tensor.matmul(dn[0:64, :], ones_sb[:, :],
                                             e_h[:, jj],
                                             start=first, stop=last)
                            nc.tensor.matmul(dn[64:128, :], ones_sb[:, :],
                                             e_h1[:, jj],
                                             start=first, stop=last)
                    r = rp.tile([128, 512], f32, tag="r")
                    nc.vector.reciprocal_approx_fast(out=r[:], in_=dn[:])
                    nc.vector.tensor_mul(avT_sb[:, m, isl], av[:], r[:])

        # ---- phase 3: output projection (partial; host sums over 2 cores) ----
        with tc.tile_pool(name="ps3", bufs=2, space="PSUM") as ps3, \
             tc.tile_pool(name="og", bufs=3) as ogp:
            for sc in range(SC):
                ssl = slice(sc * 128, (sc + 1) * 128)
                po = ps3.tile([128, 2, 512], f32, tag="po")
                for k2 in range(MC):
                    for nh in range(2):
                        nc.tensor.matmul(po[:, nh], avT_sb[:, k2, ssl],
                                         woT_sb[:, k2, nh * 512:(nh + 1) * 512],
                                         start=(k2 == 0), stop=(k2 == MC - 1))
                og = ogp.tile([128, D], f32, tag="og")
                nc.vector.tensor_copy(og[:], po[:])
                nc.sync.dma_start(out_d.ap()[ssl, :], og[:])

    nc.compile()
    return nc


def _get_nc():
    if "nc" not in _NC_CACHE:
        _NC_CACHE["nc"] = _build_nc()
    return _NC_CACHE["nc"]


def kernel(x, Wq, bq, Wk, bk, Wv, bv, Wo, bo):
    from concourse.bass_utils import run_bass_kernel_spmd

    x = np.asarray(x, dtype=np.float32)
    Wq = np.asarray(Wq, dtype=np.float32)
    Wk = np.asarray(Wk, dtype=np.float32)
    Wv = np.asarray(Wv, dtype=np.float32)
    Wo = np.asarray(Wo, dtype=np.float32)
    bq = np.asarray(bq, dtype=np.float32)
    bk = np.asarray(bk, dtype=np.float32)
    bv = np.asarray(bv, dtype=np.float32)
    bo = np.asarray(bo, dtype=np.float32)

    nc = _get_nc()

    in_maps = []
    for c in range(8):
        b = c // 2
        g = c % 2
        sl = slice(g * HD, (g + 1) * HD)
        in_maps.append({
            "xT": np.ascontiguousarray(x[b].T).astype(BF16),
            "wqT": np.ascontiguousarray((Wq[sl] * SCALE).T).astype(BF16),
            "wkT": np.ascontiguousarray(Wk[sl].T).astype(BF16),
            "wvT": np.ascontiguousarray(Wv[sl].T).astype(BF16),
            "woT": np.ascontiguousarray(Wo[:, sl].T).astype(BF16),
            "bq": np.ascontiguousarray((bq[sl] * SCALE).reshape(MC, 128).T),
            "bk": np.ascontiguousarray(bk[sl].reshape(MC, 128).T),
            "bv": bv[sl].reshape(1, HD).astype(np.float32),
        })

    _NC_CACHE["last_in_maps"] = in_maps
    res = run_bass_kernel_spmd(nc, in_maps, core_ids=list(range(8)))
    outs = [res.results[c]["out"] for c in range(8)]
    out = np.stack([outs[2 * b] + outs[2 * b + 1] for b in range(NB)])
    out = out + bo[None, None, :]
    return out.astype(np.float32)


# revision 4
# speedup vs baseline: 1.0288x; 1.0288x over previous
"""Trainium2 Bass kernel for multi-head attention (B=4, S=2048, D=1024, H=16).

Sharding: 8 cores = 4-way batch x 2-way head-group (8 heads per core).
Each core computes, for its batch element b and head-group g:
  qT/kT = (W.T x.T) in transposed layout [local_hd, S] (head-pair chunks of 128
  partitions), V in normal layout [S, local_hd], scores^T = kT.T-free matmuls
  with row-packed head pairs (K=64 x2 concurrent), exp on ACT (no max
  subtraction; scores are provably O(1) for this problem), AV + denominator
  matmuls col-packed via tile_position, reciprocal_approx_fast, normalization
  fused into the PSUM->SBUF copy, then the output projection producing a
  partial [S, D] that the host sums across the 2 head-group cores + bias.
"""

import numpy as np
import ml_dtypes
from contextlib import ExitStack

BF16 = ml_dtypes.bfloat16

S = 2048          # sequence length
D = 1024          # model dim
DH = 64           # head dim
HL = 8            # local heads per core
HD = HL * DH      # 512 local output dims per core
NB = 4            # batch
SCALE = 1.0 / (DH ** 0.5)

KC = D // 128     # 8 contraction chunks for projections
MC = HD // 128    # 4 output-dim chunks (= head pairs) per core
IC = S // 512     # 4 query chunks of 512
JC = S // 128     # 16 key chunks of 128
SC = S // 128     # 16 output row chunks

_NC_CACHE = {}


def _build_nc():
    import concourse.bacc as bacc
    import concourse.tile as tile
    from concourse import mybir

    f32 = mybir.dt.float32
    bf16 = mybir.dt.bfloat16
    Exp = mybir.ActivationFunctionType.Exp

    nc = bacc.Bacc("TRN2", target_bir_lowering=False, debug=False)

    xT_d = nc.dram_tensor("xT", [D, S], bf16, kind="ExternalInput")
    wqT_d = nc.dram_tensor("wqT", [D, HD], bf16, kind="ExternalInput")
    wkT_d = nc.dram_tensor("wkT", [D, HD], bf16, kind="ExternalInput")
    wvT_d = nc.dram_tensor("wvT", [D, HD], bf16, kind="ExternalInput")
    woT_d = nc.dram_tensor("woT", [HD, D], bf16, kind="ExternalInput")
    bq_d = nc.dram_tensor("bq", [128, MC], f32, kind="ExternalInput")
    bk_d = nc.dram_tensor("bk", [128, MC], f32, kind="ExternalInput")
    bv_d = nc.dram_tensor("bv", [1, HD], f32, kind="ExternalInput")
    out_d = nc.dram_tensor("out", [S, D], f32, kind="ExternalOutput")

    with tile.TileContext(nc) as tc, ExitStack() as ctx:
        consts = ctx.enter_context(tc.tile_pool(name="consts", bufs=1))
        persist = ctx.enter_context(tc.tile_pool(name="persist", bufs=1))

        # ---- constant/weight loads ----
        xT_sb = consts.tile([128, KC, S], bf16)
        wqT_sb = consts.tile([128, KC, HD], bf16)
        wkT_sb = consts.tile([128, KC, HD], bf16)
        wvT_sb = consts.tile([128, KC, HD], bf16)
        woT_sb = consts.tile([128, MC, D], bf16)
        bq_sb = consts.tile([128, MC], f32)
        bk_sb = consts.tile([128, MC], f32)
        bvb_sb = consts.tile([128, HD], f32)   # bv broadcast across partitions
        ones_sb = consts.tile([128, 64], bf16)

        for k in range(KC):
            nc.sync.dma_start(xT_sb[:, k, :], xT_d.ap()[k * 128:(k + 1) * 128, :])
            nc.sync.dma_start(wqT_sb[:, k, :], wqT_d.ap()[k * 128:(k + 1) * 128, :])
            nc.sync.dma_start(wkT_sb[:, k, :], wkT_d.ap()[k * 128:(k + 1) * 128, :])
            nc.sync.dma_start(wvT_sb[:, k, :], wvT_d.ap()[k * 128:(k + 1) * 128, :])
        for k in range(MC):
            nc.sync.dma_start(woT_sb[:, k, :], woT_d.ap()[k * 128:(k + 1) * 128, :])
        nc.sync.dma_start(bq_sb[:], bq_d.ap())
        nc.sync.dma_start(bk_sb[:], bk_d.ap())
        # broadcast bv along partitions via 0-step AP
        import concourse.bass as bass
        bv_ap = bv_d.ap()
        bv_bcast = bass.AP(tensor=bv_ap.tensor, offset=bv_ap.offset,
                           ap=[[0, 128]] + [bv_ap.ap[-1]])
        nc.sync.dma_start(bvb_sb[:], bv_bcast)
        nc.vector.memset(ones_sb[:], 1.0)

        qT_sb = persist.tile([128, MC, S], bf16)
        kT_sb = persist.tile([128, MC, S], bf16)
        v_sb = persist.tile([128, JC, HD], bf16)
        avT_sb = persist.tile([128, MC, S], bf16)

        # ---- phase 1: projections ----
        with tc.tile_pool(name="ps1", bufs=3, space="PSUM") as ps1:
            # qT, kT: [local_hd, S] via lhsT=W^T chunks (stationary), rhs=x^T
            for m in range(MC):
                for i in range(IC):
                    isl = slice(i * 512, (i + 1) * 512)
                    msl = slice(m * 128, (m + 1) * 128)
                    psq = ps1.tile([128, 512], f32, tag="ps1")
                    for k in range(KC):
                        nc.tensor.matmul(psq[:], wqT_sb[:, k, msl],
                                         xT_sb[:, k, isl],
                                         start=(k == 0), stop=(k == KC - 1))
                    nc.vector.tensor_scalar_add(qT_sb[:, m, isl], psq[:],
                                                bq_sb[:, m:m + 1])
                    psk = ps1.tile([128, 512], f32, tag="ps1")
                    for k in range(KC):
                        nc.tensor.matmul(psk[:], wkT_sb[:, k, msl],
                                         xT_sb[:, k, isl],
                                         start=(k == 0), stop=(k == KC - 1))
                    nc.vector.tensor_scalar_add(kT_sb[:, m, isl], psk[:],
                                                bk_sb[:, m:m + 1])
            # V in normal layout [S, local_hd]: lhsT = x^T chunk, rhs = wv^T
            for t in range(JC):
                tsl = slice(t * 128, (t + 1) * 128)
                psv = ps1.tile([128, 512], f32, tag="ps1")
                for k in range(KC):
                    nc.tensor.matmul(psv[:], xT_sb[:, k, tsl], wvT_sb[:, k, :],
                                     start=(k == 0), stop=(k == KC - 1))
                nc.vector.tensor_add(v_sb[:, t, :], psv[:], bvb_sb[:])

        # ---- phase 2: attention ----
        with tc.tile_pool(name="st", bufs=2, space="PSUM") as stp, \
             tc.tile_pool(name="av", bufs=2, space="PSUM") as avp, \
             tc.tile_pool(name="dn", bufs=2, space="PSUM") as dnp, \
             tc.tile_pool(name="ep", bufs=4) as ep, \
             tc.tile_pool(name="rp", bufs=2) as rp:
            for m in range(MC):
                h0 = 2 * m
                for i in range(IC):
                    isl = slice(i * 512, (i + 1) * 512)
                    av = avp.tile([128, 512], f32, tag="av")
                    dn = dnp.tile([128, 512], f32, tag="dn")
                    # j-range processed in 2 blocks: per block, loop A does
                    # scores+exp (PE in 64x128 row-tiled mode the whole loop:
                    # head h on row tile T0, h+1 on T8), loop B accumulates
                    # AV + denominators (PE in 128x64 col-tiled mode: h ->
                    # psum partitions 0-63, h+1 -> 64-127). Batching same-mode
                    # matmuls avoids the per-switch TensorE drain.
                    for jb in range(2):
                        e_h = ep.tile([128, JC // 2 * 512], bf16, tag="e")
                        e_h1 = ep.tile([128, JC // 2 * 512], bf16, tag="e")
                        for ljg in range(JC // 4):
                            st_h = stp.tile([128, 2, 512], f32, tag="st")
                            st_h1 = stp.tile([128, 2, 512], f32, tag="st")
                            for jj in range(2):
                                j = jb * (JC // 2) + ljg * 2 + jj
                                jsl = slice(j * 128, (j + 1) * 128)
                                nc.tensor.matmul(st_h[:, jj],
                                                 kT_sb[0:64, m, jsl],
                                                 qT_sb[0:64, m, isl],
                                                 start=True, stop=True)
                                nc.tensor.matmul(st_h1[:, jj],
                                                 kT_sb[64:128, m, jsl],
                                                 qT_sb[64:128, m, isl],
                                                 start=True, stop=True)
                            jgsl = slice(ljg * 1024, (ljg + 1) * 1024)
                            nc.scalar.activation(e_h[:, jgsl], st_h[:], Exp)
                            nc.scalar.activation(e_h1[:, jgsl], st_h1[:], Exp)
                        for lj in range(JC // 2):
                            j = jb * (JC // 2) + lj
                            jsl = slice(lj * 512, (lj + 1) * 512)
                            first = (j == 0)
                            last = (j == JC - 1)
                            nc.tensor.matmul(av[0:64, :],
                                             v_sb[:, j, h0 * 64:(h0 + 1) * 64],
                                             e_h[:, jsl],
                                             start=first, stop=last)
                            nc.tensor.matmul(av[64:128, :],
                                             v_sb[:, j,
                                                  (h0 + 1) * 64:(h0 + 2) * 64],
                                             e_h1[:, jsl],
                                             start=first, stop=last)
                            nc.tensor.matmul(dn[0:64, :], ones_sb[:, :],
                                             e_h[:, jsl],
                                             start=first, stop=last)
                            nc.tensor.matmul(dn[64:128, :], ones_sb[:, :],
                                             e_h1[:, jsl],
                                             start=first, stop=last)
                    r = rp.tile([128, 512], f32, tag="r")
                    nc.vector.reciprocal_approx_fast(out=r[:], in_=dn[:])
                    nc.vector.tensor_mul(avT_sb[:, m, isl], av[:], r[:])

        # ---- phase 3: output projection (partial; host sums over 2 cores) ----
        with tc.tile_pool(name="ps3", bufs=2, space="PSUM") as ps3, \
             tc.tile_pool(name="og", bufs=3) as ogp:
            for sc in range(SC):
                ssl = slice(sc * 128, (sc + 1) * 128)
                po = ps3.tile([128, 2, 512], f32, tag="po")
                for k2 in range(MC):
                    for nh in range(2):
                        nc.tensor.matmul(po[:, nh], avT_sb[:, k2, ssl],
                                         woT_sb[:, k2, nh * 512:(nh + 1) * 512],
                                         start=(k2 == 0), stop=(k2 == MC - 1))
                og = ogp.tile([128, D], f32, tag="og")
                nc.vector.tensor_copy(og[:], po[:])
                nc.sync.dma_start(out_d.ap()[ssl, :], og[:])

    nc.compile()
    return nc


def _get_nc():
    if "nc" not in _NC_CACHE:
        _NC_CACHE["nc"] = _build_nc()
    return _NC_CACHE["nc"]


def kernel(x, Wq, bq, Wk, bk, Wv, bv, Wo, bo):
    from concourse.bass_utils import run_bass_kernel_spmd

    x = np.asarray(x, dtype=np.float32)
    Wq = np.asarray(Wq, dtype=np.float32)
    Wk = np.asarray(Wk, dtype=np.float32)
    Wv = np.asarray(Wv, dtype=np.float32)
    Wo = np.asarray(Wo, dtype=np.float32)
    bq = np.asarray(bq, dtype=np.float32)
    bk = np.asarray(bk, dtype=np.float32)
    bv = np.asarray(bv, dtype=np.float32)
    bo = np.asarray(bo, dtype=np.float32)

    nc = _get_nc()

    in_maps = []
    for c in range(8):
        b = c // 2
        g = c % 2
        sl = slice(g * HD, (g + 1) * HD)
        in_maps.append({
            "xT": np.ascontiguousarray(x[b].T).astype(BF16),
            "wqT": np.ascontiguousarray((Wq[sl] * SCALE).T).astype(BF16),
            "wkT": np.ascontiguousarray(Wk[sl].T).astype(BF16),
            "wvT": np.ascontiguousarray(Wv[sl].T).astype(BF16),
            "woT": np.ascontiguousarray(Wo[:, sl].T).astype(BF16),
            "bq": np.ascontiguousarray((bq[sl] * SCALE).reshape(MC, 128).T),
            "bk": np.ascontiguousarray(bk[sl].reshape(MC, 128).T),
            "bv": bv[sl].reshape(1, HD).astype(np.float32),
        })

    _NC_CACHE["last_in_maps"] = in_maps
    res = run_bass_kernel_spmd(nc, in_maps, core_ids=list(range(8)))
    outs = [res.results[c]["out"] for c in range(8)]
    out = np.stack([outs[2 * b] + outs[2 * b + 1] for b in range(NB)])
    out = out + bo[None, None, :]
    return out.astype(np.float32)


# revision 11
# speedup vs baseline: 1.0437x; 1.0144x over previous
"""Trainium2 Bass kernel for multi-head attention (B=4, S=2048, D=1024, H=16).

Sharding: 8 cores = 4-way batch x 2-way head-group (8 heads per core).
Each core computes, for its batch element b and head-group g:
  qT/kT = (W.T x.T) in transposed layout [local_hd, S] (head-pair chunks of 128
  partitions), V in normal layout [S, local_hd], scores^T = kT.T-free matmuls
  with row-packed head pairs (K=64 x2 concurrent), exp on ACT (no max
  subtraction; scores are provably O(1) for this problem), AV + denominator
  matmuls col-packed via tile_position, reciprocal_approx_fast, normalization
  fused into the PSUM->SBUF copy, then the output projection producing a
  partial [S, D] that the host sums across the 2 head-group cores + bias.
"""

import numpy as np
import ml_dtypes
from contextlib import ExitStack

BF16 = ml_dtypes.bfloat16

S = 2048          # sequence length
D = 1024          # model dim
DH = 64           # head dim
HL = 8            # local heads per core
HD = HL * DH      # 512 local output dims per core
NB = 4            # batch
SCALE = 1.0 / (DH ** 0.5)

KC = D // 128     # 8 contraction chunks for projections
MC = HD // 128    # 4 output-dim chunks (= head pairs) per core
IC = S // 512     # 4 query chunks of 512
JC = S // 128     # 16 key chunks of 128
SC = S // 128     # 16 output row chunks

_NC_CACHE = {}


def _build_nc():
    import concourse.bacc as bacc
    import concourse.tile as tile
    from concourse import mybir

    f32 = mybir.dt.float32
    bf16 = mybir.dt.bfloat16
    Exp = mybir.ActivationFunctionType.Exp

    nc = bacc.Bacc("TRN2", target_bir_lowering=False, debug=False)

    xT_d = nc.dram_tensor("xT", [D, S], bf16, kind="ExternalInput")
    wqT_d = nc.dram_tensor("wqT", [D, HD], bf16, kind="ExternalInput")
    wkT_d = nc.dram_tensor("wkT", [D, HD], bf16, kind="ExternalInput")
    wvT_d = nc.dram_tensor("wvT", [D, HD], bf16, kind="ExternalInput")
    woT_d = nc.dram_tensor("woT", [HD, D], bf16, kind="ExternalInput")
    bq_d = nc.dram_tensor("bq", [128, MC], f32, kind="ExternalInput")
    bk_d = nc.dram_tensor("bk", [128, MC], f32, kind="ExternalInput")
    bv_d = nc.dram_tensor("bv", [1, HD], f32, kind="ExternalInput")
    out_d = nc.dram_tensor("out", [S, D], f32, kind="ExternalOutput")

    with tile.TileContext(nc) as tc, ExitStack() as ctx:
        import concourse.bass as bass

        consts = ctx.enter_context(tc.tile_pool(name="consts", bufs=1))
        persist = ctx.enter_context(tc.tile_pool(name="persist", bufs=1))

        woT_sb = consts.tile([128, MC, D], bf16)
        for k in range(MC):
            nc.sync.dma_start(woT_sb[:, k, :], woT_d.ap()[k * 128:(k + 1) * 128, :])

        qT_sb = persist.tile([128, MC, S], bf16)
        kT_sb = persist.tile([128, MC, S], bf16)
        # V layout per (key-chunk, local head): a 128-col block. Even local
        # heads store [V_h(64) | ones(64)], odd heads [ones(64) | V_h(64)].
        # The AV matmul lhsT is then one contiguous block and one matmul
        # produces both the attention output rows and replicated softmax
        # denominator rows.
        v_m = persist.tile([128, JC, HL, 128], bf16)
        avT_sb = persist.tile([128, MC, S], bf16)

        # ---- phase 1: projections (inputs scoped to this phase) ----
        with tc.tile_pool(name="xw1", bufs=1) as xw1, \
             tc.tile_pool(name="ps1", bufs=3, space="PSUM") as ps1:
            xT_sb = xw1.tile([128, KC, S], bf16)
            wqT_sb = xw1.tile([128, KC, HD], bf16)
            wkT_sb = xw1.tile([128, KC, HD], bf16)
            wvT_sb = xw1.tile([128, KC, HD], bf16)
            bq_sb = xw1.tile([128, MC], f32)
            bk_sb = xw1.tile([128, MC], f32)
            bvb_sb = xw1.tile([128, HD], f32)  # bv broadcast across partitions

            for k in range(KC):
                nc.sync.dma_start(xT_sb[:, k, :],
                                  xT_d.ap()[k * 128:(k + 1) * 128, :])
                nc.sync.dma_start(wqT_sb[:, k, :],
                                  wqT_d.ap()[k * 128:(k + 1) * 128, :])
                nc.sync.dma_start(wkT_sb[:, k, :],
                                  wkT_d.ap()[k * 128:(k + 1) * 128, :])
                nc.sync.dma_start(wvT_sb[:, k, :],
                                  wvT_d.ap()[k * 128:(k + 1) * 128, :])
            nc.sync.dma_start(bq_sb[:], bq_d.ap())
            nc.sync.dma_start(bk_sb[:], bk_d.ap())
            # broadcast bv along partitions via 0-step AP
            bv_ap = bv_d.ap()
            bv_bcast = bass.AP(tensor=bv_ap.tensor, offset=bv_ap.offset,
                               ap=[[0, 128]] + [bv_ap.ap[-1]])
            nc.sync.dma_start(bvb_sb[:], bv_bcast)
            # qT, kT: [local_hd, S] via lhsT=W^T chunks (stationary), rhs=x^T
            for m in range(MC):
                for i in range(IC):
                    isl = slice(i * 512, (i + 1) * 512)
                    msl = slice(m * 128, (m + 1) * 128)
                    psq = ps1.tile([128, 512], f32, tag="ps1")
                    for k in range(KC):
                        nc.tensor.matmul(psq[:], wqT_sb[:, k, msl],
                                         xT_sb[:, k, isl],
                                         start=(k == 0), stop=(k == KC - 1))
                    nc.vector.tensor_scalar_add(qT_sb[:, m, isl], psq[:],
                                                bq_sb[:, m:m + 1])
                    psk = ps1.tile([128, 512], f32, tag="ps1")
                    for k in range(KC):
                        nc.tensor.matmul(psk[:], wkT_sb[:, k, msl],
                                         xT_sb[:, k, isl],
                                         start=(k == 0), stop=(k == KC - 1))
                    nc.vector.tensor_scalar_add(kT_sb[:, m, isl], psk[:],
                                                bk_sb[:, m:m + 1])
            # V in normal layout [S, local_hd]: lhsT = x^T chunk, rhs = wv^T
            nc.vector.memset(v_m[:], 1.0)  # ones blocks; V overwrites its own
            bvb_r = bvb_sb[:].rearrange("p (h e) -> p h e", h=HL)
            for t in range(JC):
                tsl = slice(t * 128, (t + 1) * 128)
                psv = ps1.tile([128, 512], f32, tag="ps1")
                for k in range(KC):
                    nc.tensor.matmul(psv[:], xT_sb[:, k, tsl], wvT_sb[:, k, :],
                                     start=(k == 0), stop=(k == KC - 1))
                psv_r = psv[:].rearrange("p (h e) -> p h e", h=HL)
                # even heads -> cols 0-63 of their block, odd -> cols 64-127
                nc.vector.tensor_add(v_m[:, t, 0::2, 0:64],
                                     psv_r[:, 0::2, :], bvb_r[:, 0::2, :])
                nc.vector.tensor_add(v_m[:, t, 1::2, 64:128],
                                     psv_r[:, 1::2, :], bvb_r[:, 1::2, :])

        # ---- phase 2: attention ----
        with tc.tile_pool(name="st", bufs=2, space="PSUM") as stp, \
             tc.tile_pool(name="av", bufs=2, space="PSUM") as avp, \
             tc.tile_pool(name="ep", bufs=4) as ep, \
             tc.tile_pool(name="rp", bufs=4) as rp:
            for m in range(MC):
                h0 = 2 * m
                for i in range(IC):
                    isl = slice(i * 512, (i + 1) * 512)
                    avh = avp.tile([128, 512], f32, tag="av")
                    avh1 = avp.tile([128, 512], f32, tag="av")
                    # j-range in 2 blocks; per block, loop A does scores+exp
                    # (PE in 64x128 row-tiled mode: head h on row tile T0,
                    # h+1 on T8), loop B accumulates AV+denominator in plain
                    # 128x128 mode. Batching same-mode matmuls avoids the
                    # per-switch TensorE drain.
                    for jb in range(2):
                        e_h = ep.tile([128, JC // 2 * 512], bf16, tag="e")
                        e_h1 = ep.tile([128, JC // 2 * 512], bf16, tag="e")
                        eoff = 0
                        ljs = list(range(jb * (JC // 2), (jb + 1) * (JC // 2)))
                        # exp groups of 3 PSUM banks (FD=1536) amortize the
                        # per-op ACT overhead
                        groups = [ljs[0:3], ljs[3:6], ljs[6:8]]
                        for grp in groups:
                            g = len(grp)
                            st_h = stp.tile([128, 3, 512], f32, tag="st")
                            st_h1 = stp.tile([128, 3, 512], f32, tag="st")
                            for gi, j in enumerate(grp):
                                jsl = slice(j * 128, (j + 1) * 128)
                                nc.tensor.matmul(st_h[:, gi],
                                                 kT_sb[0:64, m, jsl],
                                                 qT_sb[0:64, m, isl],
                                                 start=True, stop=True)
                                nc.tensor.matmul(st_h1[:, gi],
                                                 kT_sb[64:128, m, jsl],
                                                 qT_sb[64:128, m, isl],
                                                 start=True, stop=True)
                            esl = slice(eoff, eoff + g * 512)
                            nc.scalar.activation(e_h[:, esl], st_h[:, 0:g], Exp)
                            nc.scalar.activation(e_h1[:, esl], st_h1[:, 0:g],
                                                 Exp)
                            eoff += g * 512
                        for lj in range(JC // 2):
                            j = jb * (JC // 2) + lj
                            jsl = slice(lj * 512, (lj + 1) * 512)
                            first = (j == 0)
                            last = (j == JC - 1)
                            # head h: [V_h | ones] -> U at rows 0-63,
                            # denominator replicated at rows 64-127
                            nc.tensor.matmul(avh[:], v_m[:, j, h0, :],
                                             e_h[:, jsl],
                                             start=first, stop=last)
                            # head h+1: [ones | V_h1] -> denominator at
                            # rows 0-63, U at rows 64-127
                            nc.tensor.matmul(avh1[:], v_m[:, j, h0 + 1, :],
                                             e_h1[:, jsl],
                                             start=first, stop=last)
                    # epilogue: gather denominators (lane-aligned copies),
                    # reciprocal, partition-swap halves via SBUF->SBUF DMA,
                    # then normalize fused into the PSUM->SBUF copy.
                    dcomb = rp.tile([128, 512], f32, tag="r")
                    nc.vector.tensor_copy(dcomb[64:128, :], avh[64:128, :])
                    nc.vector.tensor_copy(dcomb[0:64, :], avh1[0:64, :])
                    rcomb = rp.tile([128, 512], f32, tag="r")
                    nc.vector.reciprocal_approx_fast(out=rcomb[:],
                                                     in_=dcomb[:])
                    rswap = rp.tile([128, 512], f32, tag="r")
                    nc.sync.dma_start(rswap[0:64, :], rcomb[64:128, :])
                    nc.sync.dma_start(rswap[64:128, :], rcomb[0:64, :])
                    nc.vector.tensor_mul(avT_sb[0:64, m, isl], avh[0:64, :],
                                         rswap[0:64, :])
                    nc.vector.tensor_mul(avT_sb[64:128, m, isl],
                                         avh1[64:128, :], rswap[64:128, :])

        # ---- phase 3: output projection (partial; host sums over 2 cores) ----
        with tc.tile_pool(name="ps3", bufs=2, space="PSUM") as ps3, \
             tc.tile_pool(name="og", bufs=3) as ogp:
            for sc in range(SC):
                ssl = slice(sc * 128, (sc + 1) * 128)
                po = ps3.tile([128, 2, 512], f32, tag="po")
                for k2 in range(MC):
                    for nh in range(2):
                        nc.tensor.matmul(po[:, nh], avT_sb[:, k2, ssl],
                                         woT_sb[:, k2, nh * 512:(nh + 1) * 512],
                                         start=(k2 == 0), stop=(k2 == MC - 1))
                og = ogp.tile([128, D], f32, tag="og")
                nc.vector.tensor_copy(og[:], po[:])
                nc.sync.dma_start(out_d.ap()[ssl, :], og[:])

    nc.compile()
    return nc


def _get_nc():
    if "nc" not in _NC_CACHE:
        _NC_CACHE["nc"] = _build_nc()
    return _NC_CACHE["nc"]


def kernel(x, Wq, bq, Wk, bk, Wv, bv, Wo, bo):
    from concourse.bass_utils import run_bass_kernel_spmd

    x = np.asarray(x, dtype=np.float32)
    Wq = np.asarray(Wq, dtype=np.float32)
    Wk = np.asarray(Wk, dtype=np.float32)
    Wv = np.asarray(Wv, dtype=np.float32)
    Wo = np.asarray(Wo, dtype=np.float32)
    bq = np.asarray(bq, dtype=np.float32)
    bk = np.asarray(bk, dtype=np.float32)
    bv = np.asarray(bv, dtype=np.float32)
    bo = np.asarray(bo, dtype=np.float32)

    nc = _get_nc()

    in_maps = []
    for c in range(8):
        b = c // 2
        g = c % 2
        sl = slice(g * HD, (g + 1) * HD)
        in_maps.append({
            "xT": np.ascontiguousarray(x[b].T).astype(BF16),
            "wqT": np.ascontiguousarray((Wq[sl] * SCALE).T).astype(BF16),
            "wkT": np.ascontiguousarray(Wk[sl].T).astype(BF16),
            "wvT": np.ascontiguousarray(Wv[sl].T).astype(BF16),
            "woT": np.ascontiguousarray(Wo[:, sl].T).astype(BF16),
            "bq": np.ascontiguousarray((bq[sl] * SCALE).reshape(MC, 128).T),
            "bk": np.ascontiguousarray(bk[sl].reshape(MC, 128).T),
            "bv": bv[sl].reshape(1, HD).astype(np.float32),
        })

    _NC_CACHE["last_in_maps"] = in_maps
    res = run_bass_kernel_spmd(nc, in_maps, core_ids=list(range(8)))
    outs = [res.results[c]["out"] for c in range(8)]
    out = np.stack([outs[2 * b] + outs[2 * b + 1] for b in range(NB)])
    out = out + bo[None, None, :]
    return out.astype(np.float32)
